# revision 3
# baseline (speedup 1.0000x reference)
"""Causal self-attention (B=4, T=2048, C=1024, H=16, D=64) on 8 TRN2 NeuronCores.

Sharding: core c handles batch b = c//2 and head-half hh = c%2 (8 of 16 heads,
i.e. 512 of 1024 channels of Q/K/V and 512 rows of w_proj). Each core computes
its partial c_proj output [T, C]; the host sums the two partials per batch and
adds b_proj (tensor-parallel unshard).

Device kernel (per core, identical SPMD program):
  1. QKV projection. Q^T and K^T are produced head-dim-major ([cq, t], for the
     S^T matmuls); V is produced token-major ([t, cv]) with a ones column
     appended per head (fused softmax-denominator trick).
  2. Per head, per 512-query block: S^T[k, q] = K_h Q_h^T (contraction over
     d=64), exp via ScalarE with the 1/sqrt(D) scale folded in, causal handled
     block-sparsely (only k-tiles <= query block are computed / consumed; the
     single straddling 128x128 diagonal chunk is masked multiplicatively).
  3. PV per 128-query chunk: psum[q, 0:64] = sum_k P^T[k,q] V[k,d],
     psum[q, 64] = rowsum (ones column) -> normalize with per-partition
     reciprocal (native tensor_scalar).
  4. Y [t, cq] -> PE transpose -> Y^T [cq, t]; partial = Y^T.T @ w_proj_local.
All matmuls run in bf16 with fp32 PSUM accumulation.
"""

import sys

for _p in ("/opt/trn_rl_repo", "/root/.axon_site"):
    if _p not in sys.path:
        sys.path.append(_p)

import numpy as np
import ml_dtypes

BF16 = ml_dtypes.bfloat16

B, T, C, H = 4, 2048, 1024, 16
D = C // H          # 64
NCORES = 8
HL = H // 2         # 8 local heads
CL = HL * D         # 512 local qkv channels
P = 128
TT = T // P         # 16 token tiles
QB = 512            # query block for S^T / exp
NQB = T // QB       # 4

_BUILT = None


def _build():
    import concourse.mybir as mybir
    import concourse.tile as tile
    from concourse import bacc
    from concourse.masks import make_identity, make_upper_triangular

    bf = mybir.dt.bfloat16
    f32 = mybir.dt.float32

    nc = bacc.Bacc("TRN2", target_bir_lowering=False, debug=False,
                   num_devices=NCORES)

    xT_d = nc.dram_tensor("xT", [C // P, P, T], bf, kind="ExternalInput")
    wqk_d = nc.dram_tensor("wqk", [C // P, P, 2 * CL], bf, kind="ExternalInput")
    wv_d = nc.dram_tensor("wv", [C // P, P, CL], bf, kind="ExternalInput")
    bqk_d = nc.dram_tensor("bqk", [P, (2 * CL) // P], f32, kind="ExternalInput")
    bv_d = nc.dram_tensor("bv", [1, CL], bf, kind="ExternalInput")
    wp_d = nc.dram_tensor("wp", [CL // P, P, C], bf, kind="ExternalInput")
    out_d = nc.dram_tensor("out", [TT, P, C], f32, kind="ExternalOutput")

    NCT = C // P          # 8 contraction tiles
    NQKT = (2 * CL) // P  # 8 q/k channel tiles

    with tile.TileContext(nc) as tc:
        with (
            tc.tile_pool(name="const", bufs=1) as const,
            tc.tile_pool(name="weights", bufs=1) as wpool,
            tc.tile_pool(name="acts", bufs=1) as apool,
            tc.tile_pool(name="strips", bufs=2) as spool,
            tc.tile_pool(name="small", bufs=8) as small,
            tc.tile_pool(name="outsb", bufs=3) as opool,
            tc.tile_pool(name="ps2", bufs=2, space="PSUM") as ps2,
            tc.tile_pool(name="psy", bufs=2, space="PSUM") as psy,
            tc.tile_pool(name="pst", bufs=2, space="PSUM") as pst,
        ):
            # constants
            ident = const.tile([P, P], bf, tag="ident")
            make_identity(nc, ident[:])
            umask = const.tile([P, P], bf, tag="umask")
            make_upper_triangular(nc, umask[:], val=1.0, diag=True)
            ones1 = const.tile([1, P], bf, tag="ones1")
            nc.vector.memset(ones1[:], 1.0)
            bqk_sb = const.tile([P, NQKT], f32, tag="bqk")
            nc.sync.dma_start(out=bqk_sb[:], in_=bqk_d.ap())
            bv_sb = const.tile([1, CL], bf, tag="bv")
            nc.sync.dma_start(out=bv_sb[:], in_=bv_d.ap())

            # weights + activations
            wqk_sb = []
            wv_sb = []
            xT_sb = []
            for i in range(NCT):
                t1 = wpool.tile([P, 2 * CL], bf, tag=f"wqk{i}")
                nc.sync.dma_start(out=t1[:], in_=wqk_d.ap()[i])
                wqk_sb.append(t1)
                t2 = wpool.tile([P, CL], bf, tag=f"wv{i}")
                nc.sync.dma_start(out=t2[:], in_=wv_d.ap()[i])
                wv_sb.append(t2)
                t3 = apool.tile([P, T], bf, tag=f"xT{i}")
                nc.sync.dma_start(out=t3[:], in_=xT_d.ap()[i])
                xT_sb.append(t3)
            wp_sb = []
            for j in range(CL // P):
                t4 = wpool.tile([P, C], bf, tag=f"wp{j}")
                nc.sync.dma_start(out=t4[:], in_=wp_d.ap()[j])
                wp_sb.append(t4)

            # ---- QKV projection ----
            # Q^T / K^T head-dim-major: qkT[i] rows = local channels i*128..
            qkT_sb = [apool.tile([P, T], bf, tag=f"qkT{i}", name=f"qkT{i}") for i in range(NQKT)]
            for i in range(NQKT):
                for tch in range(T // 512):
                    ps = ps2.tile([P, 1024], f32, tag="mm", name="mm")
                    for kc in range(NCT):
                        nc.tensor.matmul(
                            ps[:, 0:512],
                            lhsT=wqk_sb[kc][:, i * P:(i + 1) * P],
                            rhs=xT_sb[kc][:, tch * 512:(tch + 1) * 512],
                            start=(kc == 0), stop=(kc == NCT - 1),
                        )
                    nc.vector.tensor_scalar_add(
                        qkT_sb[i][:, tch * 512:(tch + 1) * 512],
                        ps[:, 0:512],
                        bqk_sb[:, i:i + 1],
                    )

            # V token-major with ones column per head: v65[ti] = [128, 8*65]
            v65_sb = [apool.tile([P, HL * (D + 1)], bf, tag=f"v65{ti}", name=f"v65{ti}")
                      for ti in range(TT)]
            for ti in range(TT):
                ps = ps2.tile([P, 1024], f32, tag="mm", name="mm")
                for kc in range(NCT):
                    nc.tensor.matmul(
                        ps[:, 0:512],
                        lhsT=xT_sb[kc][:, ti * P:(ti + 1) * P],
                        rhs=wv_sb[kc][:],
                        start=(kc == 0), stop=False,
                    )
                nc.tensor.matmul(
                    ps[:, 0:512],
                    lhsT=ones1[:, 0:P],
                    rhs=bv_sb[:],
                    start=False, stop=True,
                )
                v3 = v65_sb[ti][:].rearrange("p (h e) -> p h e", e=D + 1)
                nc.vector.tensor_copy(
                    out=v3[:, :, 0:D],
                    in_=ps[:, 0:512].rearrange("p (h e) -> p h e", e=D),
                )
                nc.vector.memset(v3[:, :, D:D + 1], 1.0)

            # ---- attention ----
            y_sb = [apool.tile([P, CL], bf, tag=f"y{ti}", name=f"y{ti}") for ti in range(TT)]
            for h in range(HL):
                hp = (h % 2) * D
                qt_tile = qkT_sb[h // 2]
                kt_tile = qkT_sb[(2 * CL // P) // 2 + h // 2]
                for qb in range(NQB):
                    nkt = (qb + 1) * (QB // P)
                    strip = spool.tile([P, TT * 512], bf, tag="strip", name="strip")
                    for k2 in range(nkt // 2):
                        ps = ps2.tile([P, 1024], f32, tag="mm", name="mm")
                        for half in range(2):
                            kt = 2 * k2 + half
                            nc.tensor.matmul(
                                ps[:, half * 512:half * 512 + 512],
                                lhsT=kt_tile[hp:hp + D, kt * P:(kt + 1) * P],
                                rhs=qt_tile[hp:hp + D, qb * QB:(qb + 1) * QB],
                                start=True, stop=True,
                            )
                        nc.scalar.activation(
                            out=strip[:, k2 * 1024:(k2 + 1) * 1024],
                            in_=ps[:, 0:1024],
                            func=mybir.ActivationFunctionType.Exp,
                            scale=float(1.0 / np.sqrt(D)),
                        )
                    for qs in range(QB // P):
                        qt = (QB // P) * qb + qs
                        # mask the straddling diagonal chunk (k-tile == qt)
                        dchunk = strip[:, qt * 512 + qs * P: qt * 512 + (qs + 1) * P]
                        nc.vector.tensor_tensor(
                            out=dchunk, in0=dchunk, in1=umask[:],
                            op=mybir.AluOpType.mult,
                        )
                        ps_y = psy.tile([P, D + 1], f32, tag="y", name="psy")
                        for kt in range(qt + 1):
                            nc.tensor.matmul(
                                ps_y[:],
                                lhsT=strip[:, kt * 512 + qs * P: kt * 512 + (qs + 1) * P],
                                rhs=v65_sb[kt][:, h * (D + 1):(h + 1) * (D + 1)],
                                start=(kt == 0), stop=(kt == qt),
                            )
                        rs = small.tile([P, 1], f32, tag="rs", name="rs")
                        nc.vector.reciprocal(rs[:], ps_y[:, D:D + 1])
                        nc.vector.tensor_scalar_mul(
                            y_sb[qt][:, h * D:(h + 1) * D],
                            ps_y[:, 0:D],
                            rs[:],
                        )

            # ---- transpose Y -> Y^T ----
            yT_sb = [apool.tile([P, T], bf, tag=f"yT{j}", name=f"yT{j}") for j in range(CL // P)]
            for ti in range(TT):
                for j in range(CL // P):
                    ps_t = pst.tile([P, P], bf, tag="t", name="pst")
                    nc.tensor.transpose(
                        ps_t[:], y_sb[ti][:, j * P:(j + 1) * P], ident[:],
                    )
                    nc.vector.tensor_copy(
                        out=yT_sb[j][:, ti * P:(ti + 1) * P], in_=ps_t[:],
                    )

            # ---- output projection (partial) ----
            for ti in range(TT):
                for co in range(C // 512):
                    ps = ps2.tile([P, 1024], f32, tag="mm", name="mm")
                    for j in range(CL // P):
                        nc.tensor.matmul(
                            ps[:, 0:512],
                            lhsT=yT_sb[j][:, ti * P:(ti + 1) * P],
                            rhs=wp_sb[j][:, co * 512:(co + 1) * 512],
                            start=(j == 0), stop=(j == CL // P - 1),
                        )
                    osb = opool.tile([P, 512], f32, tag="o", name="osb")
                    nc.vector.tensor_copy(out=osb[:], in_=ps[:, 0:512])
                    nc.sync.dma_start(
                        out=out_d.ap()[ti][:, co * 512:(co + 1) * 512],
                        in_=osb[:],
                    )

    nc.compile()
    return nc


def _get_nc():
    global _BUILT
    if _BUILT is None:
        _BUILT = _build()
    return _BUILT


def _shard_inputs(x, w_attn, b_attn, w_proj):
    in_maps = []
    for c in range(NCORES):
        b, hh = divmod(c, 2)
        hoff = hh * CL
        xT = np.ascontiguousarray(x[b].T).astype(BF16).reshape(C // P, P, T)
        wqk = np.ascontiguousarray(
            np.concatenate(
                [w_attn[:, hoff:hoff + CL], w_attn[:, C + hoff:C + hoff + CL]],
                axis=1,
            )
        ).astype(BF16).reshape(C // P, P, 2 * CL)
        wv = np.ascontiguousarray(
            w_attn[:, 2 * C + hoff:2 * C + hoff + CL]
        ).astype(BF16).reshape(C // P, P, CL)
        bqk = np.ascontiguousarray(
            np.concatenate(
                [b_attn[hoff:hoff + CL], b_attn[C + hoff:C + hoff + CL]]
            ).astype(np.float32).reshape((2 * CL) // P, P).T
        )
        bv = b_attn[2 * C + hoff:2 * C + hoff + CL].astype(BF16).reshape(1, CL)
        wp = np.ascontiguousarray(
            w_proj[hoff:hoff + CL]
        ).astype(BF16).reshape(CL // P, P, C)
        in_maps.append(
            {"xT": xT, "wqk": wqk, "wv": wv, "bqk": bqk, "bv": bv, "wp": wp}
        )
    return in_maps


def _run(in_maps, trace=False):
    from concourse.bass_utils import run_bass_kernel_spmd

    nc = _get_nc()
    return run_bass_kernel_spmd(
        nc, in_maps, core_ids=list(range(NCORES)), trace=trace
    )


def kernel(x, w_attn, b_attn, w_proj, b_proj):
    x = np.asarray(x, dtype=np.float32)
    w_attn = np.asarray(w_attn, dtype=np.float32)
    b_attn = np.asarray(b_attn, dtype=np.float32)
    w_proj = np.asarray(w_proj, dtype=np.float32)
    b_proj = np.asarray(b_proj, dtype=np.float32)

    in_maps = _shard_inputs(x, w_attn, b_attn, w_proj)
    res = _run(in_maps)
    parts = [res.results[c]["out"].reshape(T, C) for c in range(NCORES)]
    out = np.stack(
        [parts[2 * b] + parts[2 * b + 1] + b_proj for b in range(B)]
    ).astype(np.float32)
    return out


# revision 7
# speedup vs baseline: 1.0891x; 1.0891x over previous
"""Causal self-attention (B=4, T=2048, C=1024, H=16, D=64) on 8 TRN2 NeuronCores.

Sharding: core c handles batch b = c//2 and head-half hh = c%2 (8 of 16 heads,
i.e. 512 of 1024 channels of Q/K/V and 512 rows of w_proj). Each core computes
its partial c_proj output [T, C]; the host sums the two partials per batch and
adds b_proj (tensor-parallel unshard).

Device kernel (per core, identical SPMD program):
  1. QKV projection. Q^T and K^T are produced head-dim-major ([cq, t], for the
     S^T matmuls); V is produced token-major ([t, cv]) with a ones column
     appended per head (fused softmax-denominator trick).
  2. Per head, per 512-query block: S^T[k, q] = K_h Q_h^T (contraction over
     d=64), exp via ScalarE with the 1/sqrt(D) scale folded in, causal handled
     block-sparsely (only k-tiles <= query block are computed / consumed; the
     single straddling 128x128 diagonal chunk is masked multiplicatively).
  3. PV per 128-query chunk: psum[q, 0:64] = sum_k P^T[k,q] V[k,d],
     psum[q, 64] = rowsum (ones column) -> normalize with per-partition
     reciprocal (native tensor_scalar).
  4. Y [t, cq] -> PE transpose -> Y^T [cq, t]; partial = Y^T.T @ w_proj_local.
All matmuls run in bf16 with fp32 PSUM accumulation.
"""

import sys

for _p in ("/opt/trn_rl_repo", "/root/.axon_site"):
    if _p not in sys.path:
        sys.path.append(_p)

import numpy as np
import ml_dtypes

BF16 = ml_dtypes.bfloat16

B, T, C, H = 4, 2048, 1024, 16
D = C // H          # 64
NCORES = 8
HL = H // 2         # 8 local heads
CL = HL * D         # 512 local qkv channels
P = 128
TT = T // P         # 16 token tiles
QB = 512            # query block for S^T / exp
NQB = T // QB       # 4

_BUILT = None


def _build():
    import concourse.mybir as mybir
    import concourse.tile as tile
    from concourse import bacc
    from concourse.masks import make_identity, make_upper_triangular

    bf = mybir.dt.bfloat16
    f32 = mybir.dt.float32

    nc = bacc.Bacc("TRN2", target_bir_lowering=False, debug=False,
                   num_devices=NCORES)

    xT_d = nc.dram_tensor("xT", [C // P, P, T], bf, kind="ExternalInput")
    wqk_d = nc.dram_tensor("wqk", [C // P, P, 2 * CL], bf, kind="ExternalInput")
    wv_d = nc.dram_tensor("wv", [C // P, P, CL], bf, kind="ExternalInput")
    bqk_d = nc.dram_tensor("bqk", [P, (2 * CL) // P], f32, kind="ExternalInput")
    bv_d = nc.dram_tensor("bv", [1, CL], bf, kind="ExternalInput")
    wp_d = nc.dram_tensor("wp", [CL // P, P, C], bf, kind="ExternalInput")
    out_d = nc.dram_tensor("out", [TT, P, C], f32, kind="ExternalOutput")

    NCT = C // P          # 8 contraction tiles
    NQKT = (2 * CL) // P  # 8 q/k channel tiles

    with tile.TileContext(nc) as tc:
        with (
            tc.tile_pool(name="const", bufs=1) as const,
            tc.tile_pool(name="weights", bufs=1) as wpool,
            tc.tile_pool(name="acts", bufs=1) as apool,
            tc.tile_pool(name="strips", bufs=2) as spool,
            tc.tile_pool(name="small", bufs=4) as small,
            tc.tile_pool(name="outsb", bufs=3) as opool,
            tc.tile_pool(name="ps2", bufs=3, space="PSUM") as ps2,
            tc.tile_pool(name="psy", bufs=2, space="PSUM") as psy,
        ):
            # constants
            ident = const.tile([P, P], bf, tag="ident")
            make_identity(nc, ident[:])
            umask = const.tile([P, P], bf, tag="umask")
            make_upper_triangular(nc, umask[:], val=1.0, diag=True)
            ones1 = const.tile([1, P], bf, tag="ones1")
            nc.vector.memset(ones1[:], 1.0)
            bqk_sb = const.tile([P, NQKT], f32, tag="bqk")
            nc.sync.dma_start(out=bqk_sb[:], in_=bqk_d.ap())
            bv_sb = const.tile([1, CL], bf, tag="bv")
            nc.sync.dma_start(out=bv_sb[:], in_=bv_d.ap())

            # weights + activations
            wqk_sb = []
            wv_sb = []
            xT_sb = []
            for i in range(NCT):
                t1 = wpool.tile([P, 2 * CL], bf, tag=f"wqk{i}")
                nc.sync.dma_start(out=t1[:], in_=wqk_d.ap()[i])
                wqk_sb.append(t1)
                t2 = wpool.tile([P, CL], bf, tag=f"wv{i}")
                nc.sync.dma_start(out=t2[:], in_=wv_d.ap()[i])
                wv_sb.append(t2)
                t3 = apool.tile([P, T], bf, tag=f"xT{i}")
                nc.sync.dma_start(out=t3[:], in_=xT_d.ap()[i])
                xT_sb.append(t3)
            wp_sb = []
            for j in range(CL // P):
                t4 = wpool.tile([P, C], bf, tag=f"wp{j}")
                nc.sync.dma_start(out=t4[:], in_=wp_d.ap()[j])
                wp_sb.append(t4)

            # ---- QKV projection ----
            # Q^T / K^T head-dim-major: qkT[i] rows = local channels i*128..
            qkT_sb = [apool.tile([P, T], bf, tag=f"qkT{i}", name=f"qkT{i}") for i in range(NQKT)]
            for i in range(NQKT):
                for tch in range(T // 512):
                    ps = ps2.tile([P, 1024], f32, tag="mm", name="mm")
                    for kc in range(NCT):
                        nc.tensor.matmul(
                            ps[:, 0:512],
                            lhsT=wqk_sb[kc][:, i * P:(i + 1) * P],
                            rhs=xT_sb[kc][:, tch * 512:(tch + 1) * 512],
                            start=(kc == 0), stop=(kc == NCT - 1),
                        )
                    nc.vector.tensor_scalar_add(
                        qkT_sb[i][:, tch * 512:(tch + 1) * 512],
                        ps[:, 0:512],
                        bqk_sb[:, i:i + 1],
                    )

            # V token-major with ones column per head: v65[ti] = [128, 8*65]
            v65_sb = [apool.tile([P, HL * (D + 1)], bf, tag=f"v65{ti}", name=f"v65{ti}")
                      for ti in range(TT)]
            for ti in range(TT):
                ps = ps2.tile([P, 1024], f32, tag="mm", name="mm")
                for kc in range(NCT):
                    nc.tensor.matmul(
                        ps[:, 0:512],
                        lhsT=xT_sb[kc][:, ti * P:(ti + 1) * P],
                        rhs=wv_sb[kc][:],
                        start=(kc == 0), stop=False,
                    )
                nc.tensor.matmul(
                    ps[:, 0:512],
                    lhsT=ones1[:, 0:P],
                    rhs=bv_sb[:],
                    start=False, stop=True,
                )
                v3 = v65_sb[ti][:].rearrange("p (h e) -> p h e", e=D + 1)
                nc.vector.tensor_copy(
                    out=v3[:, :, 0:D],
                    in_=ps[:, 0:512].rearrange("p (h e) -> p h e", e=D),
                )
                nc.vector.memset(v3[:, :, D:D + 1], 1.0)

            # ---- attention ----
            # yT[j] holds Y^T (channel-major): rows = local y channels
            # j*128.., cols = tokens. Written directly by the normalize op.
            yT_sb = [apool.tile([P, T], bf, tag=f"yT{j}", name=f"yT{j}")
                     for j in range(CL // P)]
            for h in range(HL):
                hp = (h % 2) * D
                qt_tile = qkT_sb[h // 2]
                kt_tile = qkT_sb[(2 * CL // P) // 2 + h // 2]
                for qb in range(NQB):
                    nkt = (qb + 1) * (QB // P)
                    strip = spool.tile([P, TT * 512], bf, tag="strip", name="strip")
                    for k2 in range(nkt // 2):
                        ps = ps2.tile([P, 1024], f32, tag="mm", name="mm")
                        for half in range(2):
                            kt = 2 * k2 + half
                            nc.tensor.matmul(
                                ps[:, half * 512:half * 512 + 512],
                                lhsT=kt_tile[hp:hp + D, kt * P:(kt + 1) * P],
                                rhs=qt_tile[hp:hp + D, qb * QB:(qb + 1) * QB],
                                start=True, stop=True,
                            )
                        nc.scalar.activation(
                            out=strip[:, k2 * 1024:(k2 + 1) * 1024],
                            in_=ps[:, 0:1024],
                            func=mybir.ActivationFunctionType.Exp,
                            scale=float(1.0 / np.sqrt(D)),
                        )
                    # mask the 4 straddling diagonal chunks (k-tile == qt)
                    for qs in range(QB // P):
                        qt = (QB // P) * qb + qs
                        dchunk = strip[:, qt * 512 + qs * P: qt * 512 + (qs + 1) * P]
                        nc.vector.tensor_tensor(
                            out=dchunk, in0=dchunk, in1=umask[:],
                            op=mybir.AluOpType.mult,
                        )
                    # PV transposed: psY[0:64, q] = Y^T_h, psY[64, q] = rowsum.
                    # Diagonal k-tiles stream only their causally-valid q range.
                    ps_y = psy.tile([D + 1, QB], f32, tag="y", name="psy")
                    for kt in range(nkt):
                        m = kt - (QB // P) * qb  # >=0 only for diagonal tiles
                        q0 = max(0, m * P)
                        nc.tensor.matmul(
                            ps_y[:, q0:QB],
                            lhsT=v65_sb[kt][:, h * (D + 1):(h + 1) * (D + 1)],
                            rhs=strip[:, kt * 512 + q0: (kt + 1) * 512],
                            start=(kt == 0), stop=(kt == nkt - 1),
                        )
                    # normalize: recip of rowsum row, broadcast over the 64
                    # head-dim partitions, multiply (also evicts to bf16)
                    srow = small.tile([1, QB], f32, tag="srow", name="srow")
                    nc.vector.tensor_copy(out=srow[:], in_=ps_y[D:D + 1, 0:QB])
                    rrow = small.tile([1, QB], f32, tag="rrow", name="rrow")
                    nc.vector.reciprocal_approx_fast(rrow[:], srow[:])
                    rb = small.tile([D, QB], f32, tag="rb", name="rb")
                    nc.gpsimd.partition_broadcast(rb[:], rrow[:])
                    nc.vector.tensor_tensor(
                        out=yT_sb[h // 2][hp:hp + D, qb * QB:(qb + 1) * QB],
                        in0=ps_y[0:D, 0:QB],
                        in1=rb[:],
                        op=mybir.AluOpType.mult,
                    )

            # ---- output projection (partial) ----
            for ti in range(TT):
                for co in range(C // 512):
                    ps = ps2.tile([P, 1024], f32, tag="mm", name="mm")
                    for j in range(CL // P):
                        nc.tensor.matmul(
                            ps[:, 0:512],
                            lhsT=yT_sb[j][:, ti * P:(ti + 1) * P],
                            rhs=wp_sb[j][:, co * 512:(co + 1) * 512],
                            start=(j == 0), stop=(j == CL // P - 1),
                        )
                    osb = opool.tile([P, 512], f32, tag="o", name="osb")
                    nc.vector.tensor_copy(out=osb[:], in_=ps[:, 0:512])
                    nc.sync.dma_start(
                        out=out_d.ap()[ti][:, co * 512:(co + 1) * 512],
                        in_=osb[:],
                    )

    nc.compile()
    return nc


def _get_nc():
    global _BUILT
    if _BUILT is None:
        _BUILT = _build()
    return _BUILT


def _shard_inputs(x, w_attn, b_attn, w_proj):
    in_maps = []
    for c in range(NCORES):
        b, hh = divmod(c, 2)
        hoff = hh * CL
        xT = np.ascontiguousarray(x[b].T).astype(BF16).reshape(C // P, P, T)
        wqk = np.ascontiguousarray(
            np.concatenate(
                [w_attn[:, hoff:hoff + CL], w_attn[:, C + hoff:C + hoff + CL]],
                axis=1,
            )
        ).astype(BF16).reshape(C // P, P, 2 * CL)
        wv = np.ascontiguousarray(
            w_attn[:, 2 * C + hoff:2 * C + hoff + CL]
        ).astype(BF16).reshape(C // P, P, CL)
        bqk = np.ascontiguousarray(
            np.concatenate(
                [b_attn[hoff:hoff + CL], b_attn[C + hoff:C + hoff + CL]]
            ).astype(np.float32).reshape((2 * CL) // P, P).T
        )
        bv = b_attn[2 * C + hoff:2 * C + hoff + CL].astype(BF16).reshape(1, CL)
        wp = np.ascontiguousarray(
            w_proj[hoff:hoff + CL]
        ).astype(BF16).reshape(CL // P, P, C)
        in_maps.append(
            {"xT": xT, "wqk": wqk, "wv": wv, "bqk": bqk, "bv": bv, "wp": wp}
        )
    return in_maps


def _run(in_maps, trace=False):
    from concourse.bass_utils import run_bass_kernel_spmd

    nc = _get_nc()
    return run_bass_kernel_spmd(
        nc, in_maps, core_ids=list(range(NCORES)), trace=trace
    )


def kernel(x, w_attn, b_attn, w_proj, b_proj):
    x = np.asarray(x, dtype=np.float32)
    w_attn = np.asarray(w_attn, dtype=np.float32)
    b_attn = np.asarray(b_attn, dtype=np.float32)
    w_proj = np.asarray(w_proj, dtype=np.float32)
    b_proj = np.asarray(b_proj, dtype=np.float32)

    in_maps = _shard_inputs(x, w_attn, b_attn, w_proj)
    res = _run(in_maps)
    parts = [res.results[c]["out"].reshape(T, C) for c in range(NCORES)]
    out = np.stack(
        [parts[2 * b] + parts[2 * b + 1] + b_proj for b in range(B)]
    ).astype(np.float32)
    return out
